# revision 27
# baseline (speedup 1.0000x reference)
"""Causal self-attention (B=4, T=2048, D=1024, H=16) on 8 trn2 NeuronCores.

Sharding: tensor-parallel over heads - 2 heads per core. Each core computes
qkv projections for its 2 heads (from replicated x), causal attention, and a
partial output projection (its 128 rows of w_proj). Host sums the 8 partial
[S, D] outputs.

v3 structure - every PE matmul is full 128-mode (no PE-tiling mode switches,
no exposed drains, LDWEIGHTS always hides in the background weight buffer):
  - scores: s_h = kT.T @ qz_h where qz_h is q with the OTHER head's 64
    partition rows zeroed. Both heads share one stationary kT load.
  - per-jt software pipeline (baseline-style): scores(jt) -> exp(jt) on
    ScalarE while PE runs AV(jt-1) + filler; AV accumulates [o|den] per
    head into the two banks of one fp32 psum tile via the v|ones lhsT.
  - v blocks: PE transposes from vt staging into [v0 | 1 | v1 | 1] blocks;
    AV lhsT = the contiguous slice [65h : 65h+65] = [v_h | one].
  - QKV per 512-row sub-chunk and the previous chunk's epilogue are split
    into small filler units popped one per jt inside the attention loop.
  - epilogue per chunk: den -> [8,128] via reshape-DMA, cheap reciprocal,
    K=128 zero-padded broadcast matmuls, normalize oT in SBUF, project,
    DMA out. No end-of-kernel tail.
"""

import math

import numpy as np
import ml_dtypes

B, T, D, H = 4, 2048, 1024, 16
HD = D // H           # 64
S = B * T             # 8192
P = 128
KT = D // P           # 8 k-tiles over D
SC = S // 512         # 16 qkv sub-chunks of 512 rows
JT = T // P           # 16 j-tiles per batch
NCH = T // 512        # 4 i-chunks per batch
NT = S // P           # 64 m-tiles of 128
N_CORES = 8

BFNP = ml_dtypes.bfloat16

_CACHE = {}


def _build_nc():
    import concourse.tile as tile
    import concourse.mybir as mybir
    from concourse import bacc

    BF = mybir.dt.bfloat16
    F32 = mybir.dt.float32
    Exp = mybir.ActivationFunctionType.Exp
    SCALE = 1.0 / math.sqrt(HD)

    nc = bacc.Bacc("TRN2", num_devices=N_CORES)

    xt = nc.dram_tensor("xt", [SC, P, KT * 512], BF, kind="ExternalInput").ap()
    wq = nc.dram_tensor("wq", [D, P], BF, kind="ExternalInput").ap()
    wk = nc.dram_tensor("wk", [D, P], BF, kind="ExternalInput").ap()
    wv = nc.dram_tensor("wv", [D, P], BF, kind="ExternalInput").ap()
    wp = nc.dram_tensor("wp", [P, D], BF, kind="ExternalInput").ap()
    maskt = nc.dram_tensor("maskt", [P, P], BF, kind="ExternalInput").ap()
    ebc = nc.dram_tensor("ebc", [P, NCH * P], BF, kind="ExternalInput").ap()
    ident = nc.dram_tensor("ident", [P, P], BF, kind="ExternalInput").ap()
    out_p = nc.dram_tensor("out_p", [S, D], BF, kind="ExternalOutput").ap()

    with tile.TileContext(nc) as tc:
        with tc.tile_pool(name="singles", bufs=1) as singles:
            qz = [singles.tile([P, S], BF, name=f"qz{h}") for h in (0, 1)]
            kT_sb = singles.tile([P, S], BF)
            # v blocks per m-tile: [v0 | 1 | pad | v1@80 | 1 | pad]
            v_sb = singles.tile([P, NT, 130], BF)
            wq_sb = singles.tile([P, KT, P], BF)
            wk_sb = singles.tile([P, KT, P], BF)
            wv_sb = singles.tile([P, KT, P], BF)
            wp_sb = singles.tile([P, D], BF)
            mask_sb = singles.tile([P, P], BF)
            e_sb = singles.tile([P, NCH * P], BF)
            rb_sb = singles.tile([P, P], BF)  # rows 0-7 live, 8-127 zero
            id_sb = singles.tile([P, P], BF)

            nc.sync.dma_start(out=wq_sb, in_=wq.rearrange("(kt p) n -> p kt n", p=P))
            nc.sync.dma_start(out=wk_sb, in_=wk.rearrange("(kt p) n -> p kt n", p=P))

            def late_singles():
                nc.sync.dma_start(out=wv_sb, in_=wv.rearrange("(kt p) n -> p kt n", p=P))
                nc.sync.dma_start(out=wp_sb, in_=wp)
                nc.sync.dma_start(out=mask_sb, in_=maskt)
                nc.sync.dma_start(out=e_sb, in_=ebc)
                nc.sync.dma_start(out=id_sb, in_=ident)
            nc.vector.memset(v_sb[:, :, 64:65], 1.0)
            nc.vector.memset(v_sb[:, :, 129:130], 1.0)
            nc.vector.memset(qz[0][64:128, :], 0.0)
            nc.vector.memset(qz[1][0:64, :], 0.0)
            nc.vector.memset(rb_sb, 0.0)

            with (
                tc.tile_pool(name="xc_pool", bufs=2) as xpool,
                tc.tile_pool(name="vt_pool", bufs=2) as vtp,
                tc.tile_pool(name="p_pool", bufs=8) as ppool,
                tc.tile_pool(name="oT_pool", bufs=2) as otp,
                tc.tile_pool(name="d_pool", bufs=2) as dpool,
                tc.tile_pool(name="g_pool", bufs=4) as gpool,
                tc.tile_pool(name="ob_pool", bufs=4) as obp,
                # PSUM budget (8 banks): scores 2x2 + av 2 + q/k/v/bc/pj 2
                tc.tile_pool(name="ps_s", bufs=2, space="PSUM") as ps_s,
                tc.tile_pool(name="ps_av", bufs=1, space="PSUM") as ps_av,
                tc.tile_pool(name="ps_m", bufs=2, space="PSUM") as ps_m,
            ):
                def qkv_fetch(sc):
                    xc = xpool.tile([P, KT, 512], BF, name="xc")
                    xts = xt[sc].rearrange("p (kt n) -> p kt n", kt=KT)
                    for kt in range(KT):
                        nc.sync.dma_start(out=xc[:, kt], in_=xts[:, kt])
                    return xc

                def qkv_qk(sc, xc):
                    sl = slice(sc * 512, (sc + 1) * 512)
                    q_ps = ps_m.tile([P, 512], F32, name="q_ps", tag="psm")
                    k_ps = ps_m.tile([P, 512], F32, name="k_ps", tag="psm")
                    for kt in range(KT):
                        xk = xc[:, kt]
                        nc.tensor.matmul(q_ps, lhsT=wq_sb[:, kt], rhs=xk,
                                         start=(kt == 0), stop=(kt == KT - 1))
                        nc.tensor.matmul(k_ps, lhsT=wk_sb[:, kt], rhs=xk,
                                         start=(kt == 0), stop=(kt == KT - 1))
                    for h in (0, 1):
                        nc.vector.tensor_copy(out=qz[h][h * 64:(h + 1) * 64, sl],
                                              in_=q_ps[h * 64:(h + 1) * 64, :])
                    nc.vector.tensor_copy(out=kT_sb[:, sl], in_=k_ps)

                def qkv_v(sc, xc):
                    v_ps = ps_m.tile([P, 512], F32, name="v_ps", tag="psm")
                    for kt in range(KT):
                        nc.tensor.matmul(v_ps, lhsT=wv_sb[:, kt],
                                         rhs=xc[:, kt],
                                         start=(kt == 0), stop=(kt == KT - 1))
                    vt = vtp.tile([P, 512], BF, name="vt")
                    nc.scalar.copy(out=vt, in_=v_ps)
                    # PE transposes into [v0 | 1 | v1 | 1] blocks (keeps
                    # the DMA queues free for xc/out traffic)
                    for i in range(4):
                        mt = sc * 4 + i
                        ps_t = ps_m.tile([P, P], BF, name="ps_t", tag="psm")
                        nc.tensor.transpose(ps_t, vt[:, i * P:(i + 1) * P], id_sb)
                        nc.vector.tensor_copy(out=v_sb[:, mt, 0:64],
                                              in_=ps_t[:, 0:64])
                        nc.vector.tensor_copy(out=v_sb[:, mt, 65:129],
                                              in_=ps_t[:, 64:128])

                def qkv_units(sc, xc):
                    return [lambda: qkv_qk(sc, xc), lambda: qkv_v(sc, xc)]

                def attn_chunk(b, c, fillers):
                    base = b * T
                    i0 = base + c * 512
                    njt = 4 * c + 4
                    av = ps_av.tile([P, 1024], F32, name="av")
                    pending = []

                    def flush_av():
                        for args, kw in pending:
                            nc.tensor.matmul(*args, **kw)
                        pending.clear()

                    for jt in range(njt):
                        diag = jt >= 4 * c
                        off = jt * P - c * 512 if diag else 0
                        s_t = ps_s.tile([P, 1024], F32, name="s_t")
                        lkT = kT_sb[:, base + jt * P: base + (jt + 1) * P]
                        for h in (0, 1):
                            nc.tensor.matmul(
                                s_t[:, 512 * h + off: 512 * (h + 1)],
                                lhsT=lkT, rhs=qz[h][:, i0 + off: i0 + 512],
                                start=True, stop=True)
                        flush_av()
                        if fillers:
                            fillers.pop(0)()
                        p_t = ppool.tile([P, 1024], BF, name="p_t")
                        if off < 172:
                            nc.scalar.activation(out=p_t[:, off:1024],
                                                 in_=s_t[:, off:1024],
                                                 func=Exp, scale=SCALE)
                        else:
                            for h in (0, 1):
                                nc.scalar.activation(
                                    out=p_t[:, 512 * h + off: 512 * (h + 1)],
                                    in_=s_t[:, 512 * h + off: 512 * (h + 1)],
                                    func=Exp, scale=SCALE)
                        if diag:
                            for h in (0, 1):
                                lo = 512 * h + off
                                nc.gpsimd.tensor_mul(
                                    out=p_t[:, lo:lo + P],
                                    in0=p_t[:, lo:lo + P], in1=mask_sb)
                        for h in (0, 1):
                            lv = v_sb[:, b * JT + jt, 65 * h:65 * h + 65]
                            pending.append((
                                (av[0:65, 512 * h + off: 512 * (h + 1)],),
                                dict(lhsT=lv,
                                     rhs=p_t[:, 512 * h + off: 512 * (h + 1)],
                                     start=(jt == 0), stop=(jt == njt - 1)),
                            ))
                    flush_av()
                    while fillers:
                        fillers.pop(0)()
                    # den/oT evacuation + reciprocal in [8, 128] layout
                    dstage = dpool.tile([1, 1024], F32, name="dstage")
                    oT_cb = otp.tile([P, 512], BF, name="oT_cb")
                    for h in (0, 1):
                        nc.vector.tensor_copy(
                            out=oT_cb[h * 64:(h + 1) * 64, :],
                            in_=av[0:64, 512 * h:512 * (h + 1)])
                        nc.vector.tensor_copy(
                            out=dstage[:, h * 512:(h + 1) * 512],
                            in_=av[64:65, 512 * h:512 * (h + 1)])
                    g_cb = gpool.tile([8, P], F32, name="g_cb")
                    for h in (0, 1):
                        nc.sync.dma_start(out=g_cb[h * 4:(h + 1) * 4, :],
                                          in_=dstage[0:1, h * 512:(h + 1) * 512])

                    def epi_bc():
                        # recip here (a filler in the NEXT chunk) so the
                        # in-order DVE never blocks on the g_cb DMA latency
                        r_cb = gpool.tile([8, P], F32, name="r_cb")
                        nc.vector.reciprocal(out=r_cb, in_=g_cb)
                        # rb rows 0-7 <- bf16 recip; rows 8-127 stay zero
                        nc.vector.tensor_copy(out=rb_sb[0:8, :], in_=r_cb)
                        bc = ps_m.tile([P, 512], F32, name="bc", tag="psm")
                        for it in range(4):
                            nc.tensor.matmul(bc[:, it * P:(it + 1) * P],
                                             lhsT=e_sb[:, it * P:(it + 1) * P],
                                             rhs=rb_sb, start=True, stop=True)
                        nc.vector.tensor_mul(out=oT_cb, in0=oT_cb, in1=bc)

                    def epi_pj(i):
                        mt = (b * T + c * 512) // P + i
                        ob = obp.tile([P, D], BF, name="ob")
                        for nch in range(2):
                            pj = ps_m.tile([P, 512], F32, name="pj", tag="psm")
                            nc.tensor.matmul(
                                pj,
                                lhsT=oT_cb[:, i * P:(i + 1) * P],
                                rhs=wp_sb[:, nch * 512:(nch + 1) * 512],
                                start=True, stop=True)
                            nc.vector.tensor_copy(
                                out=ob[:, nch * 512:(nch + 1) * 512], in_=pj)
                        nc.sync.dma_start(out=out_p[mt * P:(mt + 1) * P, :],
                                          in_=ob)
                    return [epi_bc] + [lambda i=i: epi_pj(i) for i in range(4)]

                # just-in-time QKV: chunk (b,c) only needs sub-chunks
                # 0..4b+c, so only sc0 runs up front; sub-chunk 4b+c+1 is
                # projected as filler inside chunk (b,c), one step ahead of
                # first use. Epilogue of the previous chunk fills too.
                xc0 = qkv_fetch(0)
                late_singles()
                for u in qkv_units(0, xc0):
                    u()
                epi = []
                xc_next = qkv_fetch(1)
                for b in range(B):
                    for c in range(NCH):
                        fillers = []
                        sc = 4 * b + c + 1
                        if sc < SC:
                            fillers.extend(qkv_units(sc, xc_next))
                        if sc + 1 < SC:
                            fillers.append(
                                lambda s_=sc + 1: fetched.append(qkv_fetch(s_)))
                        fillers.extend(epi)
                        fetched = []
                        epi = attn_chunk(b, c, fillers)
                        if fetched:
                            xc_next = fetched[0]
                for u in epi:
                    u()

    nc.compile()
    return nc


def _host_inputs(x, w_qkv, w_proj):
    x = np.asarray(x, dtype=np.float32)
    w_qkv = np.asarray(w_qkv, dtype=np.float32)
    w_proj = np.asarray(w_proj, dtype=np.float32)

    xT = np.ascontiguousarray(x.reshape(S, D).T).astype(BFNP)
    # [sc, p, kt*512+j] = xT[kt*128+p, sc*512+j]
    xt = np.ascontiguousarray(
        xT.reshape(KT, P, SC, 512).transpose(2, 1, 0, 3).reshape(SC, P, KT * 512))
    mask = np.triu(np.ones((P, P), np.float32)).astype(BFNP)  # [j, i]: 1 if j<=i
    # bc matmul: out[m, i] = sum_r E[r, m] rb[r, i]; want rb[h(m)*4 + it, i]
    ebc = np.zeros((P, NCH, P), np.float32)
    for it in range(NCH):
        ebc[it, it, 0:64] = 1.0
        ebc[4 + it, it, 64:128] = 1.0
    ebc = ebc.reshape(P, NCH * P).astype(BFNP)
    ident = np.eye(P, dtype=np.float32).astype(BFNP)

    in_maps = []
    for core in range(N_CORES):
        cs = slice(core * P, (core + 1) * P)
        in_maps.append({
            "xt": xt,
            "wq": np.ascontiguousarray(w_qkv[:, core * P:(core + 1) * P]).astype(BFNP),
            "wk": np.ascontiguousarray(w_qkv[:, D + core * P: D + (core + 1) * P]).astype(BFNP),
            "wv": np.ascontiguousarray(w_qkv[:, 2 * D + core * P: 2 * D + (core + 1) * P]).astype(BFNP),
            "wp": np.ascontiguousarray(w_proj[cs, :]).astype(BFNP),
            "maskt": mask,
            "ebc": ebc,
            "ident": ident,
        })
    return in_maps


def run_spmd(x, w_qkv, w_proj, trace=False):
    """Compile (cached) + run on 8 cores. Returns (out [B,T,D] fp32, results)."""
    from concourse import bass_utils

    if "nc" not in _CACHE:
        _CACHE["nc"] = _build_nc()
    nc = _CACHE["nc"]

    in_maps = _host_inputs(x, w_qkv, w_proj)
    res = bass_utils.run_bass_kernel_spmd(
        nc, in_maps, core_ids=list(range(N_CORES)), trace=trace)

    acc = np.zeros((S, D), np.float32)
    for r in res.results:
        acc += np.asarray(r["out_p"]).astype(np.float32)
    return acc.reshape(B, T, D), res


def kernel(x, w_qkv, w_proj):
    out, _ = run_spmd(x, w_qkv, w_proj, trace=False)
    return out
